# revision 67
# baseline (speedup 1.0000x reference)
"""Multi-head attention (B=4, S=2048, D=1024, H=16) on 8 trn2 NeuronCores.

Sharding: core c handles batch c//2 and heads (c%2)*8 .. (c%2)*8+8.
Each core computes its partial output through the fc projection; the host
sums the two per-batch partials.

Device dataflow (per core), everything fp16 inputs / fp32 accumulate:
  1. Project k, q into head-transposed layout  khT/qhT [c, token]
  2. Project v into  vhc [token, c]  with an appended ones column
  3. Per (head-pair, q-block): scores^T = kh^T q  [k, q] in PSUM, exp via
     ACT with a per-key bias (-50 for masked/padded keys, folding the
     key-padding mask), then P~^T + denominator via a [V | 1] matmul.
  4. Softmax denominators travel through DRAM per 512-token block, get
     reciprocated on an 8-lane layout, broadcast back via
     partition-stride-0 DMA reads, and multiplied into P~^T -> ctxT.
  5. fc projection from ctxT -> partial output (fp16, host sums in fp32).

Keys are compacted on the host: masked keys (mask==1) are dropped and the
remainder zero-padded to SK=1152, cutting attention work ~44%.  The -50
exp-bias makes padded keys contribute exp(-50), which underflows to an
exact 0 in fp16.

Startup: inputs are loaded in fine-grained chunks in dependency order so
projections start while the bulk of the input DMA is still in flight.
The fc projection for the last token-half is split into a pairs-0..2
prefix (computed as attention(3) filler) and a pair-3 finish, so the
final denominator roundtrip is covered by useful PE work.
"""

import numpy as np

import concourse.bass as bass
import concourse.tile as tile
from concourse import mybir
from concourse.bass_utils import run_bass_kernel_spmd

B, S, DM = 4, 2048, 1024
NH, DEPTH = 16, 64
NCORES = 8
HPC = 8                 # heads per core
C = HPC * DEPTH         # 512 output channels per core
SK = 1152               # compacted+padded key count
KC = SK // 128          # 9 key chunks
QW = 1024               # qw half width
NQW = S // QW           # 2
DC = DM // 128          # 8 contraction chunks
NPAIR = HPC // 2        # 4 head pairs (= c-tiles of 128)
SCALE = 1.0 / 8.0       # 1/sqrt(depth)
MASK_BIAS = -50.0

F32 = mybir.dt.float32
BF16 = mybir.dt.bfloat16
FP16 = mybir.dt.float16
EXP = mybir.ActivationFunctionType.Exp


def _split_excess_waits(nc, cap_default=1, cap_evsem=2):
    """walrus in this env rejects >1 sync wait per instruction (2 for event
    semaphores); hoist excess waits onto preceding same-engine NoOps."""
    n_split = 0
    for f in nc.m.functions:
        for bb in f.blocks:
            insts = list(bb.instructions)
            out = []
            for inst in insts:
                si = inst.sync_info
                cap = cap_evsem if isinstance(inst, mybir.InstEventSemaphore) else cap_default
                if si is not None and si.on_wait and len(si.on_wait) > cap:
                    waits = list(si.on_wait)
                    extra, keep = waits[:-cap], waits[-cap:]
                    for i, w in enumerate(extra):
                        nop = mybir.InstNoOp(
                            name=f"{inst.name}_waitsplit_{i}",
                            sync_info=mybir.SyncInfo(on_wait=[w], on_update=[]),
                            bass_nofuse=True,
                            engine=inst.engine,
                        )
                        nc.register_instruction(nop, overwrite=True)
                        out.append(nop)
                    inst.sync_info = mybir.SyncInfo(on_wait=keep, on_update=list(si.on_update))
                    n_split += 1
                out.append(inst)
            if n_split:
                bb.instructions = out
    return n_split


def _emit(tc, t):
    nc = tc.nc
    from contextlib import ExitStack
    ctx = ExitStack()

    persist = ctx.enter_context(tc.tile_pool(name="persist", bufs=1))
    p_xrv = ctx.enter_context(tc.tile_pool(name="xrv", bufs=2))
    p_a = ctx.enter_context(tc.tile_pool(name="apool", bufs=6))
    p_db = ctx.enter_context(tc.tile_pool(name="dbp", bufs=2))
    p_dst = ctx.enter_context(tc.tile_pool(name="dstp", bufs=2))
    p_small = ctx.enter_context(tc.tile_pool(name="small", bufs=4))
    p_fcr = ctx.enter_context(tc.tile_pool(name="fcr", bufs=8))
    p_out = ctx.enter_context(tc.tile_pool(name="outsb", bufs=16))
    p_score = ctx.enter_context(tc.tile_pool(name="pscore", bufs=2, space="PSUM"))
    p_pv = ctx.enter_context(tc.tile_pool(name="pspv", bufs=2, space="PSUM"))
    p_fill = ctx.enter_context(tc.tile_pool(name="pfill", bufs=2, space="PSUM"))

    # persistent buffers
    wq_r = persist.tile([128, DC, C], FP16, tag="wq")
    wk_r = persist.tile([128, DC, C], FP16, tag="wk")
    wv_r = persist.tile([128, DC, C], FP16, tag="wv")
    xq_r = persist.tile([128, DC, S], FP16, tag="xq")
    xk_r = persist.tile([128, DC, SK], FP16, tag="xk")
    qhT = persist.tile([128, NPAIR, S], FP16, tag="qhT")
    # khTz: per-head stationary for scores, zero-padded to the full 128
    # contraction rows so score matmuls run in 128x128 array mode (no
    # tiling-mode switch against the 128-contraction pv/proj/fc matmuls).
    khTz = persist.tile([128, 2, NPAIR, SK], FP16, tag="khTz")
    vhc = persist.tile([128, KC, HPC, DEPTH + 1], FP16, tag="vhc")
    ctxT = persist.tile([128, NPAIR, S], FP16, tag="ctxT")
    maskb = persist.tile([128, KC], F32, tag="maskb")
    ones1 = persist.tile([128, 1], F32, tag="ones1")

    # internal DRAM for the denominator shuttle:
    # row r = pair*32 + hh*16 + qw*8 + sh*4 + j   (j in 0..3, 128 tokens each)
    d_dram = nc.dram_tensor("d_dram", (NPAIR * 32, 128), F32, kind="Internal").ap()
    dinv_dram = nc.dram_tensor("dinv_dram", (NPAIR * 32, 128), F32, kind="Internal").ap()
    dinv_flat = dinv_dram.rearrange("a b -> (a b)")
    d_view = d_dram.rearrange("(pr h q s j) f -> pr h q s (j f)", h=2, q=NQW, s=2, j=4)

    nc.sync.dma_start(maskb[:], t["maskb"])
    nc.vector.memset(ones1[:], 1.0)
    nc.vector.tensor_copy(
        vhc[:, :, :, DEPTH:DEPTH + 1],
        ones1[:].to_broadcast([128, KC, HPC, 1]),
    )
    # zero the off-head halves of khTz once; the k-projection only writes
    # the live half of each head's slice
    nc.vector.memset(khTz[64:128, 0, :, :], 0.0)
    nc.vector.memset(khTz[0:64, 1, :, :], 0.0)

    # ---- chunked input loads in dependency order, spread across three
    # engines' DMA queues so each queue drains in priority order ----
    engs = [nc.sync, nc.scalar, nc.gpsimd]
    ei = 0

    def load(dst_ap, src_ap):
        nonlocal ei
        engs[ei % 3].dma_start(dst_ap, src_ap)
        ei += 1

    wk_v = t["wkT"].rearrange("(dc p) c -> p dc c", p=128)
    wq_v = t["wqT"].rearrange("(dc p) c -> p dc c", p=128)
    wv_v = t["wvT"].rearrange("(dc p) c -> p dc c", p=128)
    xk_v = t["kcT"].rearrange("(dc p) s -> p dc s", p=128)
    xq_v = t["qT"].rearrange("(dc p) s -> p dc s", p=128)
    vview = t["vcT"].rearrange("(dc p) s -> p dc s", p=128)

    # priority order: wk+xk feed khT (needed for all of attention(0)),
    # then wq + first q half, then wv (v-proj), later q half, fc last.
    for dc in range(DC):
        load(wk_r[:, dc, :], wk_v[:, dc, :])
        load(xk_r[:, dc, :], xk_v[:, dc, :])
    for dc in range(DC):
        load(wq_r[:, dc, :], wq_v[:, dc, :])
    for tb in (0, 512):
        for dc in range(DC):
            load(xq_r[:, dc, tb:tb + 512], xq_v[:, dc, tb:tb + 512])
    for dc in range(DC):
        load(wv_r[:, dc, :], wv_v[:, dc, :])
    # (v chunks stream inside the v-proj tasks; later xq halves + fc below)

    def make_proj_task(x_r, w_r, dst, pair, tb0, tlen, pool, tag, split_k,
                       wide=False):
        def task():
            ps = pool.tile([128, 1024 if wide else 512], F32, tag=tag,
                           name=f"pj_{dst.name}_{pair}_{tb0}")
            for dc in range(DC):
                nc.tensor.matmul(ps[:, :tlen],
                                 w_r[:, dc, pair * 128:(pair + 1) * 128],
                                 x_r[:, dc, tb0:tb0 + tlen],
                                 start=(dc == 0), stop=(dc == DC - 1))
            if split_k:
                nc.vector.tensor_copy(dst[0:64, 0, pair, tb0:tb0 + tlen],
                                      ps[0:64, :tlen])
                nc.vector.tensor_copy(dst[64:128, 1, pair, tb0:tb0 + tlen],
                                      ps[64:128, :tlen])
            else:
                nc.vector.tensor_copy(dst[:, pair, tb0:tb0 + tlen], ps[:, :tlen])
        return task

    def k_tasks(pair, pool, tag, wide=False):
        return [make_proj_task(xk_r, wk_r, khTz, pair, tb0, min(512, SK - tb0),
                               pool, tag, True, wide)
                for tb0 in range(0, SK, 512)]

    def q_tasks(pair, pool, tag, wide=False):
        return [make_proj_task(xq_r, wq_r, qhT, pair, tb0, 512, pool, tag,
                               False, wide)
                for tb0 in range(0, S, 512)]

    def make_v_task(kt, pool, tag):
        def task():
            xrv = p_xrv.tile([128, DC, 128], FP16, tag="xrv", name=f"xrv_{kt}")
            nc.sync.dma_start(xrv[:], vview[:, :, kt * 128:(kt + 1) * 128])
            ps = pool.tile([128, 512], F32, tag=tag, name=f"psv_{kt}")
            for dc in range(DC):
                nc.tensor.matmul(ps[:, :C], xrv[:, dc, :], wv_r[:, dc, :],
                                 start=(dc == 0), stop=(dc == DC - 1))
            nc.vector.tensor_copy(
                vhc[:, kt, :, 0:DEPTH],
                ps[:, :C].rearrange("p (h d) -> p h d", h=HPC),
            )
        return task

    def attention(pair, fillers, qw_order=None, post_sh=None, scalar_d=None):
        """fillers: {qw: closures} consumed evenly across that qw's steps.
        post_sh: {(qw, sh): closures} emitted right after that sh's
        normalize (used to cover the final denominator roundtrips)."""
        steps = 2 * KC
        for qw in (qw_order or range(NQW)):
            filler = fillers.get(qw, [])
            n_fill = len(filler)
            step = 0
            for sh in range(2):
                q0 = qw * QW + sh * 512
                pv = [p_pv.tile([DEPTH + 1, 512], F32, tag="pv",
                                name=f"pv_{pair}_{qw}_{sh}_{hh}") for hh in range(2)]
                a_prev = a_lag2 = None
                for kc in range(KC):
                    if (n_fill and
                            step * n_fill // steps != (step + 1) * n_fill // steps):
                        filler[step * n_fill // steps]()
                    step += 1
                    # attention-critical chain outranks filler chains in the
                    # scheduler's ready heap, so the exp conveyor never waits
                    # behind an 8-matmul projection burst
                    with tc.high_priority(offset=1_000_000):
                        ps_s = p_score.tile([128, 1024], F32, tag="sc",
                                            name=f"s_{pair}_{qw}_{sh}_{kc}")
                        for hh in range(2):
                            lo = 64 * hh
                            nc.tensor.matmul(ps_s[:, 512 * hh:512 * hh + 512],
                                             khTz[lo:lo + 64, hh, pair, kc * 128:(kc + 1) * 128],
                                             qhT[lo:lo + 64, pair, q0:q0 + 512],
                                             start=True, stop=True)
                        a_t = p_a.tile([128, 1024], FP16, tag="A",
                                       name=f"A_{pair}_{qw}_{sh}_{kc}")
                        nc.scalar.activation(a_t[:], ps_s[:], EXP,
                                             bias=maskb[:, kc:kc + 1], scale=SCALE)
                        if kc >= 2:
                            for hh in range(2):
                                nc.tensor.matmul(pv[hh][:], vhc[:, kc - 2, 2 * pair + hh, :],
                                                 a_lag2[:, 512 * hh:512 * hh + 512],
                                                 start=(kc == 2), stop=False)
                    a_lag2, a_prev = a_prev, a_t
                with tc.high_priority(offset=1_000_000):
                    for kc in (KC - 2, KC - 1):
                        a_x = a_lag2 if kc == KC - 2 else a_prev
                        for hh in range(2):
                            nc.tensor.matmul(pv[hh][:], vhc[:, kc, 2 * pair + hh, :],
                                             a_x[:, 512 * hh:512 * hh + 512],
                                             start=False, stop=(kc == KC - 1))
                # stage D rows + raw ctxT, then the per-sh denominator trip
                dst = p_dst.tile([64, 512], F32, tag="dst",
                                 name=f"dst_{pair}_{qw}_{sh}")
                for hh in range(2):
                    # the last shuttle's D copies go to the by-then idle
                    # scalar engine so they run alongside the ctxT copies
                    if scalar_d == (qw, sh):
                        nc.scalar.copy(dst[32 * hh:32 * hh + 1, :],
                                       pv[hh][DEPTH:DEPTH + 1, :])
                    else:
                        nc.vector.tensor_copy(dst[32 * hh:32 * hh + 1, :],
                                              pv[hh][DEPTH:DEPTH + 1, :])
                for hh in range(2):
                    nc.vector.tensor_copy(ctxT[64 * hh:64 * hh + 64, pair, q0:q0 + 512],
                                          pv[hh][0:DEPTH, :])
                for hh in range(2):
                    nc.gpsimd.dma_start(d_view[pair, hh, qw, sh, :],
                                        dst[32 * hh:32 * hh + 1, :])
                d8 = p_small.tile([8, 128], F32, tag="d8",
                                  name=f"d8_{pair}_{qw}_{sh}")
                for hh in range(2):
                    r0 = pair * 32 + hh * 16 + qw * 8 + sh * 4
                    nc.sync.dma_start(d8[4 * hh:4 * hh + 4, :], d_dram[r0:r0 + 4, :])
                dinv8 = p_small.tile([8, 128], F32, tag="dinv8",
                                     name=f"dinv8_{pair}_{qw}_{sh}")
                nc.vector.reciprocal(dinv8[:], d8[:])
                for hh in range(2):
                    r0 = pair * 32 + hh * 16 + qw * 8 + sh * 4
                    nc.sync.dma_start(dinv_dram[r0:r0 + 4, :], dinv8[4 * hh:4 * hh + 4, :])
                db = p_db.tile([128, 512], F32, tag="db",
                               name=f"db_{pair}_{qw}_{sh}")
                for hh in range(2):
                    off = (pair * 32 + hh * 16 + qw * 8 + sh * 4) * 128
                    nc.gpsimd.dma_start(db[64 * hh:64 * hh + 64, :],
                                        dinv_flat[off:off + 512].partition_broadcast(64))
                for hh in range(2):
                    sl = ctxT[64 * hh:64 * hh + 64, pair, q0:q0 + 512]
                    nc.vector.tensor_mul(sl, sl, db[64 * hh:64 * hh + 64, :])
                if post_sh:
                    for task in post_sh.get((qw, sh), []):
                        task()

    # ---- fc task construction (emission deferred) ----
    fc_view = t["fcT"].rearrange("(pr p) e -> p pr e", p=128)
    o_view = t["o"].rearrange("(tt p) e -> p tt e", p=128)
    fcrs = {}

    def emit_fcr_loads():
        for ec in range(2):
            for pair in range(NPAIR):
                fcr = p_fcr.tile([128, 512], FP16, tag="fcr", name=f"fcr_{ec}_{pair}")
                load(fcr[:], fc_view[:, pair, ec * 512:(ec + 1) * 512])
                fcrs[(ec, pair)] = fcr

    out_tiles = {}

    def get_out_tile(tt):
        if tt not in out_tiles:
            out_tiles[tt] = p_out.tile([128, DM], FP16, tag="ot", name=f"ot_{tt}")
        return out_tiles[tt]

    def fcA01(tt, ec):
        """pairs 0+1 partial, staged fp16 into the out tile (in place)."""
        def task():
            ps = p_fill.tile([128, 512], F32, tag="fill", name=f"fcA_{tt}_{ec}")
            for pair in range(2):
                nc.tensor.matmul(ps[:], ctxT[:, pair, tt * 128:(tt + 1) * 128],
                                 fcrs[(ec, pair)][:],
                                 start=(pair == 0), stop=(pair == 1))
            ot = get_out_tile(tt)
            nc.vector.tensor_copy(ot[:, ec * 512:(ec + 1) * 512], ps[:])
        return task

    def fcB23(tt, ec, store):
        """pairs 2+3 finish: accumulate onto the staged partial, store."""
        def task():
            ps = p_fill.tile([128, 512], F32, tag="fill", name=f"fcB_{tt}_{ec}")
            for pair in (2, 3):
                nc.tensor.matmul(ps[:], ctxT[:, pair, tt * 128:(tt + 1) * 128],
                                 fcrs[(ec, pair)][:],
                                 start=(pair == 2), stop=(pair == 3))
            ot = get_out_tile(tt)
            sl = ot[:, ec * 512:(ec + 1) * 512]
            nc.vector.tensor_add(sl, ps[:], sl)
            if store:
                nc.sync.dma_start(o_view[:, tt, :], ot[:])
        return task

    def fc_full(tt, ec, store):
        """all four pairs in one PSUM accumulation, single copy out."""
        def task():
            ps = p_fill.tile([128, 512], F32, tag="fill", name=f"fcf_{tt}_{ec}")
            for pair in range(NPAIR):
                nc.tensor.matmul(ps[:], ctxT[:, pair, tt * 128:(tt + 1) * 128],
                                 fcrs[(ec, pair)][:],
                                 start=(pair == 0), stop=(pair == NPAIR - 1))
            ot = get_out_tile(tt)
            nc.vector.tensor_copy(ot[:, ec * 512:(ec + 1) * 512], ps[:])
            if store:
                nc.sync.dma_start(o_view[:, tt, :], ot[:])
        return task

    # ---- schedule ----
    # pre-attention: everything that only needs wk/xk/wq/xq-qw0 runs as a
    # DMA-paced stream; v chains (gated on the later wv/xv chunks) go to
    # the fill pool so they can't block the score-pool rotation.
    kt0 = k_tasks(0, p_score, "sc", wide=True)
    for task in kt0:
        task()
    qt0 = q_tasks(0, p_score, "sc", wide=True)
    qt0[0]()
    qt0[1]()
    for task in k_tasks(1, p_score, "sc", wide=True):
        task()
    for task in k_tasks(2, p_score, "sc", wide=True):
        task()
    for task in k_tasks(3, p_score, "sc", wide=True):
        task()
    qt1 = q_tasks(1, p_score, "sc", wide=True)
    qt1[0]()
    qt1[1]()
    for kt in range(KC):
        make_v_task(kt, p_fill, "fill")()

    # remaining xq halves + fc weights stream behind the v chunks
    for tb in (1024, 1536):
        for dc in range(DC):
            load(xq_r[:, dc, tb:tb + 512], xq_v[:, dc, tb:tb + 512])
    emit_fcr_loads()

    qt0_late = q_tasks(0, p_fill, "fill")[2:]
    qt1_late = q_tasks(1, p_fill, "fill")[2:]
    qt2 = q_tasks(2, p_fill, "fill")
    qt3 = q_tasks(3, p_fill, "fill")

    attention(0, {0: qt0_late + qt2[:2], 1: qt1_late})
    attention(1, {0: qt2[2:] + qt3[:1], 1: qt3[1:3]})
    attention(2, {0: [qt3[3]] + [fcA01(tt, ec) for tt in range(6)
                                 for ec in range(2)],
                  1: [fcA01(tt, ec) for tt in range(6, 8) for ec in range(2)]})

    # fc groups that read pair-3 ctxT may only be EMITTED after the
    # attention(3) phase that normalizes those tokens (deps are
    # program-order); tt8-15 run as full 4-pair groups during qw0, with
    # tt12-13 reserved past the sh1 normalize to cover the final
    # denominator roundtrip.  tt0-7 (split A/B) finish at the tail.
    att3_fill = {1: [],
                 0: [fc_full(tt, ec, store=(ec == 1))
                     for tt in range(8, 12) for ec in range(2)]}
    post3 = {(0, 0): [fcB23(tt, ec, store=(ec == 1))
                      for tt in range(4) for ec in range(2)],
             (0, 1): [fc_full(tt, ec, store=(ec == 1))
                      for tt in (12, 13, 14, 15) for ec in range(2)] +
                     [fcB23(tt, ec, store=(ec == 1))
                      for tt in range(4, 8) for ec in range(2)]}
    attention(3, att3_fill, qw_order=[1, 0], post_sh=post3, scalar_d=(0, 1))

    ctx.close()


_NC_CACHE = {}


def _get_nc():
    if "nc" in _NC_CACHE:
        return _NC_CACHE["nc"]
    nc = bass.Bass("TRN2", target_bir_lowering=False, debug=False)
    t = {
        "qT": nc.dram_tensor("qT", (DM, S), FP16, kind="ExternalInput").ap(),
        "kcT": nc.dram_tensor("kcT", (DM, SK), FP16, kind="ExternalInput").ap(),
        "vcT": nc.dram_tensor("vcT", (DM, SK), FP16, kind="ExternalInput").ap(),
        "wqT": nc.dram_tensor("wqT", (DM, C), FP16, kind="ExternalInput").ap(),
        "wkT": nc.dram_tensor("wkT", (DM, C), FP16, kind="ExternalInput").ap(),
        "wvT": nc.dram_tensor("wvT", (DM, C), FP16, kind="ExternalInput").ap(),
        "fcT": nc.dram_tensor("fcT", (C, DM), FP16, kind="ExternalInput").ap(),
        "maskb": nc.dram_tensor("maskb", (128, KC), F32, kind="ExternalInput").ap(),
        "o": nc.dram_tensor("o", (S, DM), FP16, kind="ExternalOutput").ap(),
    }
    with tile.TileContext(nc) as tc:
        _emit(tc, t)
    _split_excess_waits(nc)
    _NC_CACHE["nc"] = nc
    return nc


def _in_map_for_core(core, v, k, q, mask, wq, wk, wv, fc):
    b = core // 2
    hs = (core % 2) * HPC
    sel = np.nonzero(mask[b] == 0)[0]
    n = len(sel)
    assert n <= SK, f"unmasked key count {n} exceeds static SK={SK}"
    kc_ = np.zeros((SK, DM), np.float16)
    kc_[:n] = k[b][sel]
    vc_ = np.zeros((SK, DM), np.float16)
    vc_[:n] = v[b][sel]
    mb = np.full(SK, MASK_BIAS, np.float32)
    mb[:n] = 0.0
    f16 = np.float16
    return {
        "qT": np.ascontiguousarray(q[b].T.astype(f16)),
        "kcT": np.ascontiguousarray(kc_.T),
        "vcT": np.ascontiguousarray(vc_.T),
        "wqT": np.ascontiguousarray(wq[hs * DEPTH:(hs + HPC) * DEPTH].T.astype(f16)),
        "wkT": np.ascontiguousarray(wk[hs * DEPTH:(hs + HPC) * DEPTH].T.astype(f16)),
        "wvT": np.ascontiguousarray(wv[hs * DEPTH:(hs + HPC) * DEPTH].T.astype(f16)),
        "fcT": np.ascontiguousarray(fc[:, hs * DEPTH:(hs + HPC) * DEPTH].T.astype(f16)),
        "maskb": np.ascontiguousarray(mb.reshape(KC, 128).T),
    }


def kernel(v, k, q, mask, wq, wk, wv, fc, _run_kwargs=None):
    v = np.asarray(v, np.float32)
    k = np.asarray(k, np.float32)
    q = np.asarray(q, np.float32)
    mask = np.asarray(mask)
    wq = np.asarray(wq, np.float32)
    wk = np.asarray(wk, np.float32)
    wv = np.asarray(wv, np.float32)
    fc = np.asarray(fc, np.float32)

    nc = _get_nc()
    in_maps = [_in_map_for_core(c, v, k, q, mask, wq, wk, wv, fc)
               for c in range(NCORES)]
    res = run_bass_kernel_spmd(nc, in_maps, core_ids=list(range(NCORES)),
                               **(_run_kwargs or {}))
    outs = [r["o"].astype(np.float32) for r in res.results]
    full = np.stack([outs[2 * b] + outs[2 * b + 1] for b in range(B)])
    if _run_kwargs:
        kernel.last_results = res
    return full


# revision 69
# speedup vs baseline: 1.0213x; 1.0213x over previous
"""Multi-head attention (B=4, S=2048, D=1024, H=16) on 8 trn2 NeuronCores.

Sharding: core c handles batch c//2 and heads (c%2)*8 .. (c%2)*8+8.
Each core computes its partial output through the fc projection; the host
sums the two per-batch partials.

Device dataflow (per core), everything fp16 inputs / fp32 accumulate:
  1. Project k, q into head-transposed layout  khT/qhT [c, token]
  2. Project v into  vhc [token, c]  with an appended ones column
  3. Per (head-pair, q-block): scores^T = kh^T q  [k, q] in PSUM, exp via
     ACT with a per-key bias (-50 for masked/padded keys, folding the
     key-padding mask), then P~^T + denominator via a [V | 1] matmul.
  4. Softmax denominators travel through DRAM per 512-token block, get
     reciprocated on an 8-lane layout, broadcast back via
     partition-stride-0 DMA reads, and multiplied into P~^T -> ctxT.
  5. fc projection from ctxT -> partial output (fp16, host sums in fp32).

Keys are compacted on the host: masked keys (mask==1) are dropped and the
remainder zero-padded to SK=1152, cutting attention work ~44%.  The -50
exp-bias makes padded keys contribute exp(-50), which underflows to an
exact 0 in fp16.

Startup: inputs are loaded in fine-grained chunks in dependency order so
projections start while the bulk of the input DMA is still in flight.
The fc projection for the last token-half is split into a pairs-0..2
prefix (computed as attention(3) filler) and a pair-3 finish, so the
final denominator roundtrip is covered by useful PE work.
"""

import numpy as np

import concourse.bass as bass
import concourse.tile as tile
from concourse import mybir
from concourse.bass_utils import run_bass_kernel_spmd

B, S, DM = 4, 2048, 1024
NH, DEPTH = 16, 64
NCORES = 8
HPC = 8                 # heads per core
C = HPC * DEPTH         # 512 output channels per core
SK = 1152               # compacted+padded key count
KC = SK // 128          # 9 key chunks
QW = 1024               # qw half width
NQW = S // QW           # 2
DC = DM // 128          # 8 contraction chunks
NPAIR = HPC // 2        # 4 head pairs (= c-tiles of 128)
SCALE = 1.0 / 8.0       # 1/sqrt(depth)
MASK_BIAS = -50.0

F32 = mybir.dt.float32
BF16 = mybir.dt.bfloat16
FP16 = mybir.dt.float16
EXP = mybir.ActivationFunctionType.Exp


def _split_excess_waits(nc, cap_default=1, cap_evsem=2):
    """walrus in this env rejects >1 sync wait per instruction (2 for event
    semaphores); hoist excess waits onto preceding same-engine NoOps."""
    n_split = 0
    for f in nc.m.functions:
        for bb in f.blocks:
            insts = list(bb.instructions)
            out = []
            for inst in insts:
                si = inst.sync_info
                cap = cap_evsem if isinstance(inst, mybir.InstEventSemaphore) else cap_default
                if si is not None and si.on_wait and len(si.on_wait) > cap:
                    waits = list(si.on_wait)
                    extra, keep = waits[:-cap], waits[-cap:]
                    for i, w in enumerate(extra):
                        nop = mybir.InstNoOp(
                            name=f"{inst.name}_waitsplit_{i}",
                            sync_info=mybir.SyncInfo(on_wait=[w], on_update=[]),
                            bass_nofuse=True,
                            engine=inst.engine,
                        )
                        nc.register_instruction(nop, overwrite=True)
                        out.append(nop)
                    inst.sync_info = mybir.SyncInfo(on_wait=keep, on_update=list(si.on_update))
                    n_split += 1
                out.append(inst)
            if n_split:
                bb.instructions = out
    return n_split


def _emit(tc, t):
    nc = tc.nc
    from contextlib import ExitStack
    ctx = ExitStack()

    persist = ctx.enter_context(tc.tile_pool(name="persist", bufs=1))
    p_xrv = ctx.enter_context(tc.tile_pool(name="xrv", bufs=2))
    p_a = ctx.enter_context(tc.tile_pool(name="apool", bufs=5))
    p_db = ctx.enter_context(tc.tile_pool(name="dbp", bufs=2))
    p_dst = ctx.enter_context(tc.tile_pool(name="dstp", bufs=2))
    p_small = ctx.enter_context(tc.tile_pool(name="small", bufs=4))
    p_fcr = ctx.enter_context(tc.tile_pool(name="fcr", bufs=8))
    p_out = ctx.enter_context(tc.tile_pool(name="outsb", bufs=16))
    p_score = ctx.enter_context(tc.tile_pool(name="pscore", bufs=2, space="PSUM"))
    p_pv = ctx.enter_context(tc.tile_pool(name="pspv", bufs=2, space="PSUM"))
    p_fill = ctx.enter_context(tc.tile_pool(name="pfill", bufs=2, space="PSUM"))

    # persistent buffers
    wq_r = persist.tile([128, DC, C], FP16, tag="wq")
    wk_r = persist.tile([128, DC, C], FP16, tag="wk")
    wv_r = persist.tile([128, DC, C], FP16, tag="wv")
    xq_r = persist.tile([128, DC, S], FP16, tag="xq")
    xk_r = persist.tile([128, DC, SK], FP16, tag="xk")
    qhT = persist.tile([128, NPAIR, S], FP16, tag="qhT")
    # khTz: per-head stationary for scores, zero-padded to the full 128
    # contraction rows so score matmuls run in 128x128 array mode (no
    # tiling-mode switch against the 128-contraction pv/proj/fc matmuls).
    khTz = persist.tile([128, 2, NPAIR, SK], FP16, tag="khTz")
    vhc = persist.tile([128, KC, HPC, DEPTH + 1], FP16, tag="vhc")
    ctxT = persist.tile([128, NPAIR, S], FP16, tag="ctxT")
    maskb = persist.tile([128, KC], F32, tag="maskb")
    ones1 = persist.tile([128, 1], F32, tag="ones1")

    # internal DRAM for the denominator shuttle:
    # row r = pair*32 + hh*16 + qw*8 + sh*4 + j   (j in 0..3, 128 tokens each)
    d_dram = nc.dram_tensor("d_dram", (NPAIR * 32, 128), F32, kind="Internal").ap()
    dinv_dram = nc.dram_tensor("dinv_dram", (NPAIR * 32, 128), F32, kind="Internal").ap()
    dinv_flat = dinv_dram.rearrange("a b -> (a b)")
    d_view = d_dram.rearrange("(pr h q s j) f -> pr h q s (j f)", h=2, q=NQW, s=2, j=4)

    nc.sync.dma_start(maskb[:], t["maskb"])
    nc.vector.memset(ones1[:], 1.0)
    nc.vector.tensor_copy(
        vhc[:, :, :, DEPTH:DEPTH + 1],
        ones1[:].to_broadcast([128, KC, HPC, 1]),
    )
    # zero the off-head halves of khTz once; the k-projection only writes
    # the live half of each head's slice
    nc.vector.memset(khTz[64:128, 0, :, :], 0.0)
    nc.vector.memset(khTz[0:64, 1, :, :], 0.0)

    # ---- chunked input loads in dependency order, spread across three
    # engines' DMA queues so each queue drains in priority order ----
    engs = [nc.sync, nc.scalar, nc.gpsimd]
    ei = 0

    def load(dst_ap, src_ap):
        nonlocal ei
        engs[ei % 3].dma_start(dst_ap, src_ap)
        ei += 1

    wk_v = t["wkT"].rearrange("(dc p) c -> p dc c", p=128)
    wq_v = t["wqT"].rearrange("(dc p) c -> p dc c", p=128)
    wv_v = t["wvT"].rearrange("(dc p) c -> p dc c", p=128)
    xk_v = t["kcT"].rearrange("(dc p) s -> p dc s", p=128)
    xq_v = t["qT"].rearrange("(dc p) s -> p dc s", p=128)
    vview = t["vcT"].rearrange("(dc p) s -> p dc s", p=128)

    # priority order: wk+xk feed khT (needed for all of attention(0)),
    # then wq + first q half, then wv (v-proj), later q half, fc last.
    for dc in range(DC):
        load(wk_r[:, dc, :], wk_v[:, dc, :])
        load(xk_r[:, dc, :], xk_v[:, dc, :])
    for dc in range(DC):
        load(wq_r[:, dc, :], wq_v[:, dc, :])
    for tb in (0, 512):
        for dc in range(DC):
            load(xq_r[:, dc, tb:tb + 512], xq_v[:, dc, tb:tb + 512])
    for dc in range(DC):
        load(wv_r[:, dc, :], wv_v[:, dc, :])
    # (v chunks stream inside the v-proj tasks; later xq halves + fc below)

    def make_proj_task(x_r, w_r, dst, pair, tb0, tlen, pool, tag, split_k,
                       wide=False):
        def task():
            ps = pool.tile([128, 1024 if wide else 512], F32, tag=tag,
                           name=f"pj_{dst.name}_{pair}_{tb0}")
            for dc in range(DC):
                nc.tensor.matmul(ps[:, :tlen],
                                 w_r[:, dc, pair * 128:(pair + 1) * 128],
                                 x_r[:, dc, tb0:tb0 + tlen],
                                 start=(dc == 0), stop=(dc == DC - 1))
            if split_k:
                nc.vector.tensor_copy(dst[0:64, 0, pair, tb0:tb0 + tlen],
                                      ps[0:64, :tlen])
                nc.vector.tensor_copy(dst[64:128, 1, pair, tb0:tb0 + tlen],
                                      ps[64:128, :tlen])
            else:
                nc.vector.tensor_copy(dst[:, pair, tb0:tb0 + tlen], ps[:, :tlen])
        return task

    def k_tasks(pair, pool, tag, wide=False):
        return [make_proj_task(xk_r, wk_r, khTz, pair, tb0, min(512, SK - tb0),
                               pool, tag, True, wide)
                for tb0 in range(0, SK, 512)]

    def q_tasks(pair, pool, tag, wide=False):
        return [make_proj_task(xq_r, wq_r, qhT, pair, tb0, 512, pool, tag,
                               False, wide)
                for tb0 in range(0, S, 512)]

    def make_v_task(kt, pool, tag):
        def task():
            xrv = p_xrv.tile([128, DC, 128], FP16, tag="xrv", name=f"xrv_{kt}")
            nc.sync.dma_start(xrv[:], vview[:, :, kt * 128:(kt + 1) * 128])
            ps = pool.tile([128, 512], F32, tag=tag, name=f"psv_{kt}")
            for dc in range(DC):
                nc.tensor.matmul(ps[:, :C], xrv[:, dc, :], wv_r[:, dc, :],
                                 start=(dc == 0), stop=(dc == DC - 1))
            nc.vector.tensor_copy(
                vhc[:, kt, :, 0:DEPTH],
                ps[:, :C].rearrange("p (h d) -> p h d", h=HPC),
            )
        return task

    def attention(pair, fillers, qw_order=None, post_sh=None):
        """fillers: {qw: closures} consumed evenly across that qw's steps.
        post_sh: {(qw, sh): closures} emitted right after that sh's
        normalize (used to cover the final denominator roundtrips)."""
        steps = 2 * KC
        for qw in (qw_order or range(NQW)):
            filler = fillers.get(qw, [])
            n_fill = len(filler)
            step = 0
            for sh in range(2):
                q0 = qw * QW + sh * 512
                pv = [p_pv.tile([DEPTH + 1, 512], F32, tag="pv",
                                name=f"pv_{pair}_{qw}_{sh}_{hh}") for hh in range(2)]
                a_prev = a_lag2 = None
                for kc in range(KC):
                    if (n_fill and
                            step * n_fill // steps != (step + 1) * n_fill // steps):
                        filler[step * n_fill // steps]()
                    step += 1
                    # attention-critical chain outranks filler chains in the
                    # scheduler's ready heap, so the exp conveyor never waits
                    # behind an 8-matmul projection burst
                    with tc.high_priority(offset=1_000_000):
                        ps_s = p_score.tile([128, 1024], F32, tag="sc",
                                            name=f"s_{pair}_{qw}_{sh}_{kc}")
                        for hh in range(2):
                            lo = 64 * hh
                            nc.tensor.matmul(ps_s[:, 512 * hh:512 * hh + 512],
                                             khTz[lo:lo + 64, hh, pair, kc * 128:(kc + 1) * 128],
                                             qhT[lo:lo + 64, pair, q0:q0 + 512],
                                             start=True, stop=True)
                        a_t = p_a.tile([128, 1024], FP16, tag="A",
                                       name=f"A_{pair}_{qw}_{sh}_{kc}")
                        nc.scalar.activation(a_t[:], ps_s[:], EXP,
                                             bias=maskb[:, kc:kc + 1], scale=SCALE)
                        if kc >= 2:
                            for hh in range(2):
                                nc.tensor.matmul(pv[hh][:], vhc[:, kc - 2, 2 * pair + hh, :],
                                                 a_lag2[:, 512 * hh:512 * hh + 512],
                                                 start=(kc == 2), stop=False)
                    a_lag2, a_prev = a_prev, a_t
                with tc.high_priority(offset=1_000_000):
                    for kc in (KC - 2, KC - 1):
                        a_x = a_lag2 if kc == KC - 2 else a_prev
                        for hh in range(2):
                            nc.tensor.matmul(pv[hh][:], vhc[:, kc, 2 * pair + hh, :],
                                             a_x[:, 512 * hh:512 * hh + 512],
                                             start=False, stop=(kc == KC - 1))
                # stage D rows + raw ctxT, then the per-sh denominator trip
                dst = p_dst.tile([64, 512], F32, tag="dst",
                                 name=f"dst_{pair}_{qw}_{sh}")
                for hh in range(2):
                    nc.vector.tensor_copy(dst[32 * hh:32 * hh + 1, :],
                                          pv[hh][DEPTH:DEPTH + 1, :])
                for hh in range(2):
                    nc.vector.tensor_copy(ctxT[64 * hh:64 * hh + 64, pair, q0:q0 + 512],
                                          pv[hh][0:DEPTH, :])
                for hh in range(2):
                    nc.gpsimd.dma_start(d_view[pair, hh, qw, sh, :],
                                        dst[32 * hh:32 * hh + 1, :])
                d8 = p_small.tile([8, 128], F32, tag="d8",
                                  name=f"d8_{pair}_{qw}_{sh}")
                for hh in range(2):
                    r0 = pair * 32 + hh * 16 + qw * 8 + sh * 4
                    nc.sync.dma_start(d8[4 * hh:4 * hh + 4, :], d_dram[r0:r0 + 4, :])
                dinv8 = p_small.tile([8, 128], F32, tag="dinv8",
                                     name=f"dinv8_{pair}_{qw}_{sh}")
                nc.vector.reciprocal(dinv8[:], d8[:])
                for hh in range(2):
                    r0 = pair * 32 + hh * 16 + qw * 8 + sh * 4
                    nc.sync.dma_start(dinv_dram[r0:r0 + 4, :], dinv8[4 * hh:4 * hh + 4, :])
                db = p_db.tile([128, 512], F32, tag="db",
                               name=f"db_{pair}_{qw}_{sh}")
                for hh in range(2):
                    off = (pair * 32 + hh * 16 + qw * 8 + sh * 4) * 128
                    nc.gpsimd.dma_start(db[64 * hh:64 * hh + 64, :],
                                        dinv_flat[off:off + 512].partition_broadcast(64))
                for hh in range(2):
                    sl = ctxT[64 * hh:64 * hh + 64, pair, q0:q0 + 512]
                    nc.vector.tensor_mul(sl, sl, db[64 * hh:64 * hh + 64, :])
                if post_sh:
                    for task in post_sh.get((qw, sh), []):
                        task()

    # ---- fc task construction (emission deferred) ----
    fc_view = t["fcT"].rearrange("(pr p) e -> p pr e", p=128)
    o_view = t["o"].rearrange("(tt p) e -> p tt e", p=128)
    fcrs = {}

    def emit_fcr_loads():
        for ec in range(2):
            for pair in range(NPAIR):
                fcr = p_fcr.tile([128, 512], FP16, tag="fcr", name=f"fcr_{ec}_{pair}")
                load(fcr[:], fc_view[:, pair, ec * 512:(ec + 1) * 512])
                fcrs[(ec, pair)] = fcr

    out_tiles = {}

    def get_out_tile(tt):
        if tt not in out_tiles:
            out_tiles[tt] = p_out.tile([128, DM], FP16, tag="ot", name=f"ot_{tt}")
        return out_tiles[tt]

    def fcA01(tt, ec):
        """pairs 0+1 partial, staged fp16 into the out tile (in place)."""
        def task():
            ps = p_fill.tile([128, 512], F32, tag="fill", name=f"fcA_{tt}_{ec}")
            for pair in range(2):
                nc.tensor.matmul(ps[:], ctxT[:, pair, tt * 128:(tt + 1) * 128],
                                 fcrs[(ec, pair)][:],
                                 start=(pair == 0), stop=(pair == 1))
            ot = get_out_tile(tt)
            nc.vector.tensor_copy(ot[:, ec * 512:(ec + 1) * 512], ps[:])
        return task

    def fcB23(tt, ec, store):
        """pairs 2+3 finish: accumulate onto the staged partial, store."""
        def task():
            ps = p_fill.tile([128, 512], F32, tag="fill", name=f"fcB_{tt}_{ec}")
            for pair in (2, 3):
                nc.tensor.matmul(ps[:], ctxT[:, pair, tt * 128:(tt + 1) * 128],
                                 fcrs[(ec, pair)][:],
                                 start=(pair == 2), stop=(pair == 3))
            ot = get_out_tile(tt)
            sl = ot[:, ec * 512:(ec + 1) * 512]
            nc.vector.tensor_add(sl, ps[:], sl)
            if store:
                nc.sync.dma_start(o_view[:, tt, :], ot[:])
        return task

    def fc_full(tt, ec, store):
        """all four pairs in one PSUM accumulation, single copy out."""
        def task():
            ps = p_fill.tile([128, 512], F32, tag="fill", name=f"fcf_{tt}_{ec}")
            for pair in range(NPAIR):
                nc.tensor.matmul(ps[:], ctxT[:, pair, tt * 128:(tt + 1) * 128],
                                 fcrs[(ec, pair)][:],
                                 start=(pair == 0), stop=(pair == NPAIR - 1))
            ot = get_out_tile(tt)
            nc.vector.tensor_copy(ot[:, ec * 512:(ec + 1) * 512], ps[:])
            if store:
                nc.sync.dma_start(o_view[:, tt, :], ot[:])
        return task

    # ---- schedule ----
    # pre-attention: everything that only needs wk/xk/wq/xq-qw0 runs as a
    # DMA-paced stream; v chains (gated on the later wv/xv chunks) go to
    # the fill pool so they can't block the score-pool rotation.
    kt0 = k_tasks(0, p_score, "sc", wide=True)
    for task in kt0:
        task()
    qt0 = q_tasks(0, p_score, "sc", wide=True)
    qt0[0]()
    qt0[1]()
    for task in k_tasks(1, p_score, "sc", wide=True):
        task()
    for task in k_tasks(2, p_score, "sc", wide=True):
        task()
    for task in k_tasks(3, p_score, "sc", wide=True):
        task()
    qt1 = q_tasks(1, p_score, "sc", wide=True)
    qt1[0]()
    qt1[1]()
    for kt in range(KC):
        make_v_task(kt, p_fill, "fill")()

    # remaining xq halves + fc weights stream behind the v chunks
    for tb in (1024, 1536):
        for dc in range(DC):
            load(xq_r[:, dc, tb:tb + 512], xq_v[:, dc, tb:tb + 512])
    emit_fcr_loads()

    qt0_late = q_tasks(0, p_fill, "fill")[2:]
    qt1_late = q_tasks(1, p_fill, "fill")[2:]
    qt2 = q_tasks(2, p_fill, "fill")
    qt3 = q_tasks(3, p_fill, "fill")

    attention(0, {0: qt0_late + qt2[:2], 1: qt1_late})
    attention(1, {0: qt2[2:] + qt3[:1], 1: qt3[1:3]})
    attention(2, {0: [qt3[3]] + [fcA01(tt, ec) for tt in range(4)
                                 for ec in range(2)],
                  1: [fcA01(tt, ec) for tt in range(4, 6) for ec in range(2)]})

    # fc groups that read pair-3 ctxT may only be EMITTED after the
    # attention(3) phase that normalizes those tokens (deps are
    # program-order); tt8-15 run as full 4-pair groups during qw0, with
    # tt12-13 reserved past the sh1 normalize to cover the final
    # denominator roundtrip.  tt0-7 (split A/B) finish at the tail; the
    # last pair-0/1 partials are held back as attention(3) qw1 filler.
    att3_fill = {1: [fcA01(tt, ec) for tt in range(6, 8) for ec in range(2)],
                 0: [fc_full(tt, ec, store=(ec == 1))
                     for tt in range(8, 12) for ec in range(2)]}
    post3 = {(0, 0): [fcB23(tt, ec, store=(ec == 1))
                      for tt in range(4) for ec in range(2)],
             (0, 1): [fc_full(tt, ec, store=(ec == 1))
                      for tt in (12, 13, 14, 15) for ec in range(2)] +
                     [fcB23(tt, ec, store=(ec == 1))
                      for tt in range(4, 8) for ec in range(2)]}
    attention(3, att3_fill, qw_order=[1, 0], post_sh=post3)

    ctx.close()


_NC_CACHE = {}


def _get_nc():
    if "nc" in _NC_CACHE:
        return _NC_CACHE["nc"]
    nc = bass.Bass("TRN2", target_bir_lowering=False, debug=False)
    t = {
        "qT": nc.dram_tensor("qT", (DM, S), FP16, kind="ExternalInput").ap(),
        "kcT": nc.dram_tensor("kcT", (DM, SK), FP16, kind="ExternalInput").ap(),
        "vcT": nc.dram_tensor("vcT", (DM, SK), FP16, kind="ExternalInput").ap(),
        "wqT": nc.dram_tensor("wqT", (DM, C), FP16, kind="ExternalInput").ap(),
        "wkT": nc.dram_tensor("wkT", (DM, C), FP16, kind="ExternalInput").ap(),
        "wvT": nc.dram_tensor("wvT", (DM, C), FP16, kind="ExternalInput").ap(),
        "fcT": nc.dram_tensor("fcT", (C, DM), FP16, kind="ExternalInput").ap(),
        "maskb": nc.dram_tensor("maskb", (128, KC), F32, kind="ExternalInput").ap(),
        "o": nc.dram_tensor("o", (S, DM), FP16, kind="ExternalOutput").ap(),
    }
    with tile.TileContext(nc) as tc:
        _emit(tc, t)
    _split_excess_waits(nc)
    _NC_CACHE["nc"] = nc
    return nc


def _in_map_for_core(core, v, k, q, mask, wq, wk, wv, fc):
    b = core // 2
    hs = (core % 2) * HPC
    sel = np.nonzero(mask[b] == 0)[0]
    n = len(sel)
    assert n <= SK, f"unmasked key count {n} exceeds static SK={SK}"
    kc_ = np.zeros((SK, DM), np.float16)
    kc_[:n] = k[b][sel]
    vc_ = np.zeros((SK, DM), np.float16)
    vc_[:n] = v[b][sel]
    mb = np.full(SK, MASK_BIAS, np.float32)
    mb[:n] = 0.0
    f16 = np.float16
    return {
        "qT": np.ascontiguousarray(q[b].T.astype(f16)),
        "kcT": np.ascontiguousarray(kc_.T),
        "vcT": np.ascontiguousarray(vc_.T),
        "wqT": np.ascontiguousarray(wq[hs * DEPTH:(hs + HPC) * DEPTH].T.astype(f16)),
        "wkT": np.ascontiguousarray(wk[hs * DEPTH:(hs + HPC) * DEPTH].T.astype(f16)),
        "wvT": np.ascontiguousarray(wv[hs * DEPTH:(hs + HPC) * DEPTH].T.astype(f16)),
        "fcT": np.ascontiguousarray(fc[:, hs * DEPTH:(hs + HPC) * DEPTH].T.astype(f16)),
        "maskb": np.ascontiguousarray(mb.reshape(KC, 128).T),
    }


def kernel(v, k, q, mask, wq, wk, wv, fc, _run_kwargs=None):
    v = np.asarray(v, np.float32)
    k = np.asarray(k, np.float32)
    q = np.asarray(q, np.float32)
    mask = np.asarray(mask)
    wq = np.asarray(wq, np.float32)
    wk = np.asarray(wk, np.float32)
    wv = np.asarray(wv, np.float32)
    fc = np.asarray(fc, np.float32)

    nc = _get_nc()
    in_maps = [_in_map_for_core(c, v, k, q, mask, wq, wk, wv, fc)
               for c in range(NCORES)]
    res = run_bass_kernel_spmd(nc, in_maps, core_ids=list(range(NCORES)),
                               **(_run_kwargs or {}))
    outs = [r["o"].astype(np.float32) for r in res.results]
    full = np.stack([outs[2 * b] + outs[2 * b + 1] for b in range(B)])
    if _run_kwargs:
        kernel.last_results = res
    return full
